# revision 18
# baseline (speedup 1.0000x reference)
"""Trainium2 Bass kernel for nn_AttentionBlock (gnn_message_passing).

Reference computation per batch b (B=8, N=2048, T=64, Cin=16, Cout=4):
  t   = relu(conv1(X) + sigmoid(conv2(X)) + conv3(X))        # (N, 62, 4)
  si  = t.reshape(N, 248) @ fcw[:248]
  sj  = t.reshape(N, 248) @ fcw[248:]
  u   = leaky_relu(si[:, None] + sj[None, :] + fcb, 0.01)    # (N, N)
  v   = where(A != 0, u, 0)
  out = softmax(v, axis=1) * A

Sharding: data-parallel over batch, one batch per NeuronCore (8 cores),
A + weights replicated. No collectives.

Per-core kernel plan:
  * X is pre-transposed on host to (t*16+ci, node) and the three 1x3 convs
    are expressed as one banded matmul: for each 128-node tile, accumulate
    over 8 K-chunks of X^T (each (128,128)) times banded weight chunks
    (128, 496) into one PSUM bank, plus a K=1 matmul adding the biases.
    Column layout: [0:248) = conv1+conv3 output (to*4+co), [248:496) = conv2.
  * t = relu(y13 + sigmoid(y2)) on ACT/DVE; si/sj via fused
    tensor_tensor_reduce against broadcast fcw.
  * sj column -> DRAM -> row -> broadcast to all 128 partitions via a K=1
    ones-matmul into PSUM (128, 2048).
  * Attention per 128-row tile: ACT Lrelu(sj_b + si) -> DVE mask-mul by A ->
    ACT Exp with accum_out (free row-sum) -> DVE reciprocal/scale ->
    final mask-mul -> DMA out.  Softmax max-subtraction is skipped: scores
    are bounded (|v| < ~8) so exp is safe in fp32 and softmax is
    shift-invariant.
"""

import os
import numpy as np

N = 2048
T = 64
CIN = 16
COUT = 4
TO = T - 2          # 62
D = TO * COUT       # 248
NB = 8              # cores / batches
KCH = 8             # K-chunks of X^T (1024 = 8*128)
NT = N // 128       # 16 node/row tiles

# packed constant block column offsets
C_XT = 0
C_WB = C_XT + KCH * N              # 16384
C_WIJ = C_WB + KCH * 2 * D         # 20352
C_BROW = C_WIJ + 2 * D             # 20848
C_ONES = C_BROW + 2 * D            # 21344
C_TOT = C_ONES + 128               # 21472

_cache = {}


def _build_program(fcb_val: float):
    import concourse.mybir as mybir
    from concourse import bacc, tile

    f32 = mybir.dt.float32
    f32r = mybir.dt.float32r
    AF = mybir.ActivationFunctionType
    OP = mybir.AluOpType

    # Bacc (not raw Bass): its compile pipeline splits sync waits to the
    # 1-per-instruction TRN2 limit and moves matmul waits to ldweights.
    nc = bacc.Bacc("TRN2", target_bir_lowering=False, debug=False)

    # single packed constant block: one DMA -> one semaphore wait downstream
    # cols [0,16384): X^T (8 chunks x 2048 nodes)
    # [16384,20352): banded conv weights (8 x 496)
    # [20352,20848): fcw broadcast (wij, used as f32)
    # [20848,21344): bias row (row 0 only)
    # [21344,21472): ones row (row 0 only)
    cst_d = nc.dram_tensor("cst", [128, C_TOT], f32r, kind="ExternalInput")
    a_d = nc.dram_tensor("a", [N, N], f32, kind="ExternalInput")
    out_d = nc.dram_tensor("out", [N, N], f32, kind="ExternalOutput")

    with tile.TileContext(nc) as tc:
        with (
            tc.tile_pool(name="const", bufs=1) as cpool,
            tc.tile_pool(name="apool", bufs=3) as apool,
            tc.tile_pool(name="upool", bufs=3) as upool,
            tc.tile_pool(name="small", bufs=2) as spool,
            tc.tile_pool(name="stat", bufs=4) as stpool,
            tc.tile_pool(name="psum_y", bufs=2, space="PSUM") as ppool,
            tc.tile_pool(name="psum_sj", bufs=1, space="PSUM") as pjpool,
            tc.tile_pool(name="dram", bufs=1, space="DRAM") as dpool,
        ):
            # ---- constant load (single DMA) ----
            cst_sb = cpool.tile([128, C_TOT], f32r)
            nc.sync.dma_start(cst_sb[:], cst_d[:])

            def xt_sb(i, nt):
                c = C_XT + i * N + nt * 128
                return cst_sb[:, c: c + 128]

            def wb_sb(i):
                c = C_WB + i * 2 * D
                return cst_sb[:, c: c + 2 * D]

            wij_sb = cst_sb[:, C_WIJ: C_WIJ + 2 * D].bitcast(f32)
            brow_sb = cst_sb[0:1, C_BROW: C_BROW + 2 * D]
            ones_sb = cst_sb[0:1, C_ONES: C_ONES + 128]

            si_col = cpool.tile([128, NT], f32)   # si per row-tile column
            sj_col = cpool.tile([128, NT], f32)

            # ---- phase 1: conv + si/sj per node tile ----
            for nt in range(NT):
                y = ppool.tile([128, 2 * D], f32)     # one PSUM bank (1984B)
                for i in range(KCH):
                    nc.tensor.matmul(
                        y[:],
                        lhsT=xt_sb(i, nt),
                        rhs=wb_sb(i),
                        start=(i == 0),
                        stop=False,
                    )
                nc.tensor.matmul(
                    y[:],
                    lhsT=ones_sb,
                    rhs=brow_sb,
                    start=False,
                    stop=True,
                )
                sg = spool.tile([128, D], f32)
                nc.scalar.activation(sg[:], y[:, D: 2 * D], AF.Sigmoid)
                t2 = spool.tile([128, D], f32)
                nc.vector.tensor_tensor(t2[:], y[:, 0:D], sg[:], op=OP.add)
                tr = spool.tile([128, D], f32)
                nc.scalar.activation(tr[:], t2[:], AF.Relu)
                # si = sum(tr*wi) (+fcb after loop) ; sj = sum(tr*wj)
                # (tensor_tensor_reduce is unavailable at runtime here)
                pq = spool.tile([128, 2 * D], f32)
                nc.vector.tensor_tensor(pq[:, 0:D], tr[:], wij_sb[:, 0:D],
                                        op=OP.mult)
                nc.vector.tensor_tensor(pq[:, D: 2 * D], tr[:],
                                        wij_sb[:, D: 2 * D], op=OP.mult)
                nc.vector.tensor_reduce(
                    si_col[:, nt: nt + 1], pq[:, 0:D],
                    axis=mybir.AxisListType.X, op=OP.add,
                )
                nc.vector.tensor_reduce(
                    sj_col[:, nt: nt + 1], pq[:, D: 2 * D],
                    axis=mybir.AxisListType.X, op=OP.add,
                )

            # fold fcb into si
            nc.vector.tensor_scalar_add(si_col[:], si_col[:], fcb_val)

            # ---- phase 2: sj column -> row -> broadcast ----
            sj_dram = dpool.tile([N], f32)
            nc.sync.dma_start(
                sj_dram.rearrange("(c p) -> p c", p=128), sj_col[:]
            )
            sj_row = cpool.tile([1, N], f32r)
            nc.sync.dma_start(
                sj_row[:], sj_dram.rearrange("(o n) -> o n", o=1).bitcast(f32r)
            )
            sj_b = pjpool.tile([128, N], f32)     # 4 PSUM banks
            for q in range(4):
                nc.tensor.matmul(
                    sj_b[:, q * 512: (q + 1) * 512],
                    lhsT=ones_sb,
                    rhs=sj_row[:, q * 512: (q + 1) * 512],
                    start=True,
                    stop=True,
                )

            # ---- phase 3: attention rows ----
            for rt in range(NT):
                a_t = apool.tile([128, N], f32)
                nc.sync.dma_start(a_t[:], a_d[rt * 128: (rt + 1) * 128, :])
                u = upool.tile([128, N], f32)
                # u = lrelu(sj + si + fcb)   (fcb folded into si_col)
                nc.scalar.activation(
                    u[:], sj_b[:], AF.Lrelu,
                    bias=si_col[:, rt: rt + 1], scale=1.0, alpha=0.01,
                )
                nc.vector.tensor_tensor(u[:], u[:], a_t[:], op=OP.mult)
                s = stpool.tile([128, 1], f32)
                nc.scalar.activation(u[:], u[:], AF.Exp, accum_out=s[:])
                r = stpool.tile([128, 1], f32)
                nc.vector.reciprocal(r[:], s[:])
                nc.vector.tensor_scalar_mul(u[:], u[:], r[:])
                o = upool.tile([128, N], f32)
                nc.vector.tensor_tensor(o[:], u[:], a_t[:], op=OP.mult)
                nc.sync.dma_start(out_d[rt * 128: (rt + 1) * 128, :], o[:])

    nc.finalize()   # Bacc.compile(): wait splitting, reg alloc, event sems
    return nc


def _host_prep(X, A, cw1, cb1, cw2, cb2, cw3, cb3, fcw, fcb):
    B = X.shape[0]

    # banded weights: Wbig (1024, 496); col to*4+co = conv1+conv3, D+ = conv2
    W13 = (cw1 + cw3)[:, :, 0, :]     # (4, 16, 3)
    W2 = cw2[:, :, 0, :]
    Wbig = np.zeros((T * CIN, 2 * D), np.float32)
    for to in range(TO):
        for k in range(3):
            t = to + k
            Wbig[t * CIN: (t + 1) * CIN, to * 4: (to + 1) * 4] += W13[:, :, k].T
            Wbig[t * CIN: (t + 1) * CIN, D + to * 4: D + (to + 1) * 4] += W2[:, :, k].T
    wb = Wbig.reshape(KCH, 128, 2 * D).transpose(1, 0, 2).reshape(128, KCH * 2 * D)

    cst = np.zeros((128, C_TOT), np.float32)
    cst[:, C_WB: C_WB + KCH * 2 * D] = wb
    cst[:, C_WIJ: C_WIJ + 2 * D] = fcw[None, :].astype(np.float32)
    cst[0, C_BROW: C_BROW + D] = np.tile(cb1 + cb3, TO)
    cst[0, C_BROW + D: C_BROW + 2 * D] = np.tile(cb2, TO)
    cst[0, C_ONES: C_ONES + 128] = 1.0

    a_full = np.ascontiguousarray(A.astype(np.float32))

    in_maps = []
    for b in range(B):
        c = cst.copy()
        # X^T per batch: rows r = t*16+ci; chunk i = r//128, partition = r%128
        c[:, C_XT: C_XT + KCH * N] = (
            X[b].reshape(N, T * CIN).T.reshape(KCH, 128, N)
            .transpose(1, 0, 2).reshape(128, KCH * N)
        )
        in_maps.append({"cst": c, "a": a_full})
    return in_maps


def kernel(X, A, cw1, cb1, cw2, cb2, cw3, cb3, fcw, fcb, _trace=False):
    X = np.asarray(X, np.float32)
    A = np.asarray(A, np.float32)
    cw1 = np.asarray(cw1, np.float32); cb1 = np.asarray(cb1, np.float32)
    cw2 = np.asarray(cw2, np.float32); cb2 = np.asarray(cb2, np.float32)
    cw3 = np.asarray(cw3, np.float32); cb3 = np.asarray(cb3, np.float32)
    fcw = np.asarray(fcw, np.float32)
    fcb_val = float(np.asarray(fcb, np.float32))

    from concourse.bass_utils import run_bass_kernel_spmd

    key = ("prog", round(fcb_val, 9))
    if key not in _cache:
        _cache[key] = _build_program(fcb_val)
    nc = _cache[key]

    in_maps = _host_prep(X, A, cw1, cb1, cw2, cb2, cw3, cb3, fcw, fcb)
    res = run_bass_kernel_spmd(
        nc, in_maps, core_ids=list(range(NB)), trace=_trace,
    )
    kernel.last_results = res
    out = np.stack([res.results[b]["out"] for b in range(NB)], axis=0)
    return out.astype(np.float32)


kernel.last_results = None
